# revision 39
# baseline (speedup 1.0000x reference)
"""MoE feed-forward block (B=2, T=2048, D=1024, FF=4096, E=8, top-2) on 8 trn2 cores.

Strategy (expert-parallel, matching the sharding hint):
  - Router (x @ Wr.T, top-2, softmax) computed on host in fp64: it is tiny
    and its output is *indices* + weights, i.e. the dispatch.
  - Dispatch: tokens are gathered per expert on host (the all-to-all), padded
    to a common capacity, and each of the 8 cores runs the FFN of one expert
    over its routed tokens.
  - Combine: host does out[idx_e] += w_e * y_e (fp32), the weighted
    scatter-add, then reshapes to [B, T, D].

Device kernel: both GEMMs run on the PE in fp8 (e4m3) DoubleRow perf mode,
which contracts K=256 per instruction at 0.5 cycles/row -- 4x the fp16 MAC
rate. Plain e4m3 quantization costs ~2.7% error per operand, so (token,
expert) pairs are split into two precision classes by their softmax combine
weight w:

  precise (w >= TAU): each GEMM uses a compensated 3-product split
      A @ B ~= A_hi @ B_hi + A_lo @ B_hi + A_hi @ B_lo   (hi/lo both e4m3,
      shared power-of-2 scale, lo = quantized residual; ~7e-4 per operand)
      at 0.75x the fp16 cycle count.
  cheap (w < TAU): single-product pure fp8 at 0.25x the fp16 cycle count.
      Its ~5% FFN error enters the output scaled by w < TAU, and only a
      ~quarter of pairs land here, so the end-to-end error stays ~1.2e-2,
      under the 2e-2 gate.

Layouts (pair dim = the DoubleRow K-pair, i.e. k-blocks 2j/2j+1):
  GEMM1 (h = gelu(x @ W1)): psum[f128, ctile] += W1p[j][128,2,f128].T xp[j]
    [128,2,ctile]. Precise: 4 j-tiles x 3 products; ACT gelu -> fp16 h16,
    Pool casts h_hi (e4m3), DVE forms h_lo = h16 - h_hi. Cheap: 4 matmuls,
    ACT gelu straight to e4m3 hq. W1 scaled by 1024 on host; descaled by
    the ACT `scale` operand.
  GEMM2 (y = h @ W2): psum[c128, dtile] += hp[j2][128,2,c128].T W2p[j2]
    [128,2,dtile]; precise 3 products, cheap 1. The h pair APs slice
    per-c-chunk [128, 32, clen] SBUF tiles (pair stride = clen; per-chunk
    tiles keep the pair-read dependency boxes inside one chunk). W2 scaled
    by 2048 on host; descale folded into the host-side combine weights.
  Token-count remainders (CP%128, CQ%128) run transposed (W2 stationary, h
  moving, out [d128, R]) so their matmul cost scales with R, not 128.

DMA: merged hi|lo(|cheap) tensors keep the DMA count low (HWDGE descriptor
generation, ~625ns per DMA, is a serial resource). W1 streams on the SP
queue in consumption order, W2 follows; x rides the ACT hwdge queue with
the warmup-gating piece queued last, so the unavoidable head stall (the
early working set exceeds DMA bandwidth) collapses into one gap instead of
several -- each separate PE idle gap would restart the p-state ramp (3us at
half clock). GEMM2 interleaves its many small remainder/cheap groups
between big precise blocks, and the final store is split three ways so the
copy/DGE/DMA/sem tail chain mostly overlaps matmuls.
"""

import sys

sys.path.insert(0, "/opt/trn_rl_repo")

import math
from contextlib import ExitStack

import numpy as np
import ml_dtypes

import concourse.tile as tile
from concourse import bacc, mybir
from concourse.bass_utils import run_bass_kernel_spmd

B, T, D, FF, E, TOPK = 2, 2048, 1024, 4096, 8, 2
N_CORES = 8
FC = FF // 128  # 32 f-blocks
KJ1 = D // 256  # 4 K-pair tiles in GEMM1
KJ2 = FF // 256  # 16 K-pair tiles in GEMM2
S_W1 = 1024.0  # host scale on W1 (power of 2: exact)
S_W2 = 2048.0  # host scale on W2
TAU = 0.35  # combine-weight threshold for the cheap (pure-fp8) class

E4NP = ml_dtypes.float8_e4m3

_cache: dict[tuple, object] = {}


def _c_chunks(C: int) -> list[tuple[int, int]]:
    """Split C into <=512-wide moving chunks."""
    out, off = [], 0
    while off < C:
        n = min(512, C - off)
        out.append((off, n))
        off += n
    return out


def _build(CP: int, CQ: int):
    f16 = mybir.dt.float16
    e4 = mybir.dt.float8e4

    nc = bacc.Bacc("TRN2", target_bir_lowering=False, debug=False)
    # merged tensors (one DMA each -- HWDGE descriptor generation at
    # ~625ns per DMA is a serial resource, so fewer/bigger transfers win):
    # x pairs [j, p, i, (hi CP | lo CP | cheap CQ)], [c] = x[c, (2j+i)*128+p]
    xhl = nc.dram_tensor("xhl", [KJ1, 128, 2, 2 * CP + CQ], e4, kind="ExternalInput").ap()
    # W1 pairs, f-quarter-major [(q*4+j), p, i, (hi 1024 | lo 1024)],
    # [f'] = 1024*W1[(2j+i)*128+p, q*1024+f']
    w1hl = nc.dram_tensor("w1hl", [4 * KJ1, 128, 2, 2048], e4, kind="ExternalInput").ap()
    # W2 pairs [j2, p, i, (hi 1024 | lo 1024)], [d] = 2048*W2[(2j2+i)*128+p, d]
    w2hl = nc.dram_tensor("w2hl", [KJ2, 128, 2, 2048], e4, kind="ExternalInput").ap()

    outs = {}
    for nm, Cn in (("p", CP), ("q", CQ)):
        CBn, Rn = Cn // 128, Cn % 128
        outs[f"y_{nm}"] = (
            nc.dram_tensor(f"y_{nm}", [CBn * 128, 1024], f16, kind="ExternalOutput").ap()
            if CBn
            else None
        )
        outs[f"yr_{nm}"] = (
            nc.dram_tensor(f"yr_{nm}", [8, 128, Rn], f16, kind="ExternalOutput").ap()
            if Rn
            else None
        )

    with tile.TileContext(nc) as tc:
        _emit(nc, tc, xhl, w1hl, w2hl, outs, CP, CQ)
    nc.compile()
    return nc


def _emit(nc, tc, xhl, w1hl, w2hl, outs, CP, CQ):
    f16 = mybir.dt.float16
    f32 = mybir.dt.float32
    e4 = mybir.dt.float8e4
    GELU = mybir.ActivationFunctionType.Gelu
    COPY = mybir.ActivationFunctionType.Copy
    DR = mybir.MatmulPerfMode.DoubleRow
    chunksP = _c_chunks(CP)
    chunksQ = _c_chunks(CQ)
    CPB, RP = CP // 128, CP % 128
    CQB, RQ = CQ // 128, CQ % 128
    # chunk holding each class's token remainder (never straddles: chunk
    # edges and CB*128 are both 128-aligned)
    ciRP = (CPB * 128) // 512 if RP else -1
    ciRQ = (CQB * 128) // 512 if RQ else -1

    with ExitStack() as ctx:
        xp = ctx.enter_context(tc.tile_pool(name="xp", bufs=1))
        # a full f-quarter (hi+lo merged x 4 k-tiles) must be live at
        # once; 6 bufs gives half a quarter of prefetch. Merged tiles keep
        # the DMA count low (16): HWDGE descriptor generation (~625ns per
        # DMA) is a serial resource and limits W1 supply, not bandwidth.
        w1p = ctx.enter_context(tc.tile_pool(name="w1p", bufs=8))
        w2p = ctx.enter_context(tc.tile_pool(name="w2p", bufs=1))
        hp = ctx.enter_context(tc.tile_pool(name="hp", bufs=1))
        h16p = ctx.enter_context(tc.tile_pool(name="h16p", bufs=4))
        ps1p = ctx.enter_context(tc.tile_pool(name="ps1p", bufs=5, space="PSUM"))
        ps2p = ctx.enter_context(tc.tile_pool(name="ps2p", bufs=3, space="PSUM"))
        yp = ctx.enter_context(tc.tile_pool(name="yp", bufs=4))

        # --- input DMA. W1 streams alone on the SP queue in consumption
        # order; x rides the ACT hwdge queue; W2 follows W1 on SP. The j=0
        # x tile splits so the warmup's hi chunk leads.
        XW = 2 * CP + CQ
        x_t = [xp.tile([128, 2, XW], e4, name=f"x{j}") for j in range(KJ1)]
        c0len = chunksP[0][1]
        # The warmup's gating piece (j=0 hi chunk) goes LAST among the
        # hi parts, consolidating the unavoidable DMA-deficit stall into
        # ONE gap (each separate PE idle gap would restart the p-state
        # ramp). Only the hi halves ride ahead of the gate -- the lo/cheap
        # halves aren't needed until the warmup's second phase, so they
        # follow it, pulling the PE start earlier.
        for j in range(1, KJ1):
            nc.scalar.dma_start(x_t[j][:, :, :CP], xhl[j][:, :, :CP])
        nc.scalar.dma_start(x_t[0][:, :, :c0len], xhl[0][:, :, :c0len])
        for j in range(1, KJ1):
            nc.scalar.dma_start(x_t[j][:, :, CP:], xhl[j][:, :, CP:])
        nc.scalar.dma_start(x_t[0][:, :, c0len:], xhl[0][:, :, c0len:])

        w1_t = {}
        for q in range(4):
            for j in range(KJ1):
                t = w1p.tile([128, 2, 2048], e4, tag="w1", name=f"w1_{q}_{j}")
                nc.sync.dma_start(t[:], w1hl[q * KJ1 + j])
                w1_t[q, j] = t

        w2_t = []
        for j2 in range(KJ2):
            t = w2p.tile([128, 2, 2048], e4, name=f"w2_{j2}")
            nc.sync.dma_start(t[:], w2hl[j2])
            w2_t.append(t)

        # per-c-chunk h tiles (pair-read dependency boxes stay in-chunk)
        hh_c = [hp.tile([128, FC, cl], e4, name=f"hh{ci}") for ci, (_, cl) in enumerate(chunksP)]
        hl_c = [hp.tile([128, FC, cl], e4, name=f"hl{ci}") for ci, (_, cl) in enumerate(chunksP)]
        hq_c = [hp.tile([128, FC, cl], e4, name=f"hq{ci}") for ci, (_, cl) in enumerate(chunksQ)]

        def w1_slices(fb, j, lo=False):
            t = w1_t[fb // 8, j]
            off = (1024 if lo else 0) + (fb % 8) * 128
            return t[:, :, off : off + 128]

        def g1_precise(ps, fb, coff, clen, j, first, last, skip_lo=False):
            lh = w1_slices(fb, j)
            ll = w1_slices(fb, j, lo=True)
            rh = x_t[j][:, :, coff : coff + clen]
            rl = x_t[j][:, :, CP + coff : CP + coff + clen]
            o = ps[:, :clen]
            nc.tensor.matmul(o, lh, rh, start=first, stop=False, perf_mode=DR)
            nc.tensor.matmul(o, ll, rh, start=False, stop=False, perf_mode=DR)
            if not skip_lo:
                nc.tensor.matmul(o, lh, rl, start=False, stop=last, perf_mode=DR)

        def g1_post(ps, fb, ci, clen):
            # one ACT gelu pass (fp16); Pool casts the hi part to e4m3;
            # DVE forms the residual. Spreads the work over three engines.
            h16 = h16p.tile([128, 512], f16, tag="h16", name=f"h16_{fb}_{ci}")
            nc.scalar.activation(h16[:, :clen], ps[:, :clen], GELU, scale=1.0 / S_W1)
            nc.gpsimd.tensor_copy(hh_c[ci][:, fb, :clen], h16[:, :clen])
            nc.vector.tensor_sub(
                hl_c[ci][:, fb, :clen], h16[:, :clen], hh_c[ci][:, fb, :clen]
            )

        def g1_cheap(fb):
            for ci, (coff, clen) in enumerate(chunksQ):
                ps = ps1p.tile([128, 512], f32, tag="ps1", name=f"psq_{fb}_{ci}")
                o = ps[:, :clen]
                for j in range(KJ1):
                    rq = x_t[j][:, :, 2 * CP + coff : 2 * CP + coff + clen]
                    nc.tensor.matmul(
                        o, w1_slices(fb, j), rq,
                        start=(j == 0), stop=(j == KJ1 - 1), perf_mode=DR,
                    )
                nc.scalar.activation(hq_c[ci][:, fb, :clen], o, GELU, scale=1.0 / S_W1)

        # --- GEMM1. Warmup: j-outer over the 4 f-blocks of half-quarter 0,
        # precise chunk 0, hi-products first (so only x_hi gates the start).
        warm_fb = 4
        coff0 = chunksP[0][0]
        ps_head = [
            ps1p.tile([128, 512], f32, tag="ps1", name=f"psh_{fb}")
            for fb in range(warm_fb)
        ]
        jorder = [1, 2, 3, 0]  # matches x-tile arrival order
        for ji, j in enumerate(jorder):
            for fb in range(warm_fb):
                g1_precise(ps_head[fb], fb, coff0, c0len, j,
                           first=(ji == 0), last=False, skip_lo=True)
        for ji, j in enumerate(jorder):
            for fb in range(warm_fb):
                rl = x_t[j][:, :, CP + coff0 : CP + coff0 + c0len]
                nc.tensor.matmul(
                    ps_head[fb][:, :c0len], w1_slices(fb, j), rl,
                    start=False, stop=(ji == KJ1 - 1), perf_mode=DR,
                )
        for fb in range(warm_fb):
            g1_post(ps_head[fb], fb, 0, c0len)

        # regular groups fb-major (= W1 stream order). Per fb: the cheap
        # group first (GEMM2's cheap phase runs early), then precise chunks
        # with the remainder-holding chunk first.
        idx_chunks = list(enumerate(chunksP))
        if 0 <= ciRP and len(chunksP) > 1:
            idx_chunks = [idx_chunks[ciRP]] + idx_chunks[:ciRP] + idx_chunks[ciRP + 1 :]
        for fb in range(FC):
            g1_cheap(fb)
            for ci, (coff, clen) in idx_chunks:
                if fb < warm_fb and ci == 0:
                    continue
                ps = ps1p.tile([128, 512], f32, tag="ps1", name=f"ps1_{fb}_{ci}")
                for j in range(KJ1):
                    g1_precise(ps, fb, coff, clen, j,
                               first=(j == 0), last=(j == KJ1 - 1))
                g1_post(ps, fb, ci, clen)

        # --- GEMM2. Phase order puts work whose h deps complete earliest
        # first, so the PE always has queued work while the GEMM1 ACT/Pool/
        # DVE pipeline drains: precise remainder -> cheap (blocks+remainder)
        # -> precise blocks in the remainder's chunk -> remaining blocks.
        def rem_db(hh_src, hl_src, ci, lo, R, yr, n_prod, db):
            # transposed: W2 stationary, h moving, out [d-block 128, R]
            ps = ps2p.tile([128, 512], f32, tag="ps2", name=f"psr_{db}_{n_prod}")
            o = ps[:, :R]
            for j2 in range(KJ2):
                t2 = w2_t[j2]
                lh = t2[:, :, db * 128 : (db + 1) * 128]
                ll = t2[:, :, 1024 + db * 128 : 1024 + (db + 1) * 128]
                rh = hh_src[ci][:, 2 * j2 : 2 * j2 + 2, lo : lo + R]
                nc.tensor.matmul(o, lh, rh, start=(j2 == 0),
                                 stop=(n_prod == 1 and j2 == KJ2 - 1), perf_mode=DR)
                if n_prod == 3:
                    rl = hl_src[ci][:, 2 * j2 : 2 * j2 + 2, lo : lo + R]
                    nc.tensor.matmul(o, ll, rh, start=False, stop=False, perf_mode=DR)
                    nc.tensor.matmul(o, lh, rl, start=False,
                                     stop=(j2 == KJ2 - 1), perf_mode=DR)
            yrs = yp.tile([128, R], f16, tag=f"yr{n_prod}", name=f"yr_{db}_{n_prod}", bufs=2)
            # ACT (idle once GEMM1's gelus drain) does these copies; DVE is
            # still working off the GEMM1 residual-sub queue.
            nc.scalar.activation(yrs[:], ps[:, :R], COPY)
            nc.sync.dma_start(yr[db], yrs[:])

        def g2_block(hh_src, hl_src, y_dst, chunks_, cb, doff, dlen, n_prod, tag):
            ci = (cb * 128) // 512
            cl = cb * 128 - chunks_[ci][0]
            ps = ps2p.tile([128, 512], f32, tag="ps2", name=f"ps2_{cb}_{doff}_{n_prod}{tag}")
            o = ps[:, :dlen]
            for j2 in range(KJ2):
                t2 = w2_t[j2]
                lh = hh_src[ci][:, 2 * j2 : 2 * j2 + 2, cl : cl + 128]
                rh = t2[:, :, doff : doff + dlen]
                nc.tensor.matmul(o, lh, rh, start=(j2 == 0),
                                 stop=(n_prod == 1 and j2 == KJ2 - 1), perf_mode=DR)
                if n_prod == 3:
                    ll = hl_src[ci][:, 2 * j2 : 2 * j2 + 2, cl : cl + 128]
                    rl = t2[:, :, 1024 + doff : 1024 + doff + dlen]
                    nc.tensor.matmul(o, ll, rh, start=False, stop=False, perf_mode=DR)
                    nc.tensor.matmul(o, lh, rl, start=False,
                                     stop=(j2 == KJ2 - 1), perf_mode=DR)
            ysb = yp.tile([128, 512], f16, tag="y", name=f"y_{cb}_{doff}_{n_prod}{tag}")
            nc.vector.tensor_copy(ysb[:, :dlen], o)
            nc.sync.dma_start(y_dst[cb * 128 : (cb + 1) * 128, doff : doff + dlen], ysb[:, :dlen])

        # Emission order: two big precise blocks (whose h deps complete by
        # GEMM1's end) lead; the many small remainder/cheap groups are then
        # interleaved ~2 per big block so their store-chain latencies hide
        # behind 5us of queued matmuls; the split final store goes last.
        smalls = []
        if RP:
            loP = CPB * 128 - chunksP[ciRP][0]
            smalls += [
                (lambda db=db: rem_db(hh_c, hl_c, ciRP, loP, RP, outs["yr_p"], 3, db))
                for db in range(8)
            ]
        for cqb in range(CQB):
            for doff in (0, 512):
                smalls.append(
                    lambda cqb=cqb, doff=doff: g2_block(
                        hq_c, None, outs["y_q"], chunksQ, cqb, doff, 512, 1, ""
                    )
                )
        if RQ:
            loQ = CQB * 128 - chunksQ[ciRQ][0]
            smalls += [
                (lambda db=db: rem_db(hq_c, None, ciRQ, loQ, RQ, outs["yr_q"], 1, db))
                for db in range(8)
            ]

        cbs = sorted(range(CPB), key=lambda cb: (0 if (cb * 128) // 512 == ciRP else 1, cb))
        bigs = []
        for i, cb in enumerate(cbs):
            for doff in (0, 512):
                if i == len(cbs) - 1 and doff == 512:
                    bigs.append(
                        lambda cb=cb: (
                            g2_block(hh_c, hl_c, outs["y_p"], chunksP, cb, 512, 256, 3, "a"),
                            g2_block(hh_c, hl_c, outs["y_p"], chunksP, cb, 768, 128, 3, "b"),
                            g2_block(hh_c, hl_c, outs["y_p"], chunksP, cb, 896, 128, 3, "c"),
                        )
                    )
                else:
                    bigs.append(
                        lambda cb=cb, doff=doff: g2_block(
                            hh_c, hl_c, outs["y_p"], chunksP, cb, doff, 512, 3, ""
                        )
                    )
        n_lead = min(2, len(bigs) - 1)
        for bg in bigs[:n_lead]:
            bg()
        rest = bigs[n_lead:]
        si = 0
        for k, bg in enumerate(rest):
            quota = (len(smalls) - si + len(rest) - k - 1) // max(1, len(rest) - k)
            for _ in range(quota):
                if si < len(smalls):
                    smalls[si]()
                    si += 1
            bg()
        while si < len(smalls):
            smalls[si]()
            si += 1


def _route(xf: np.ndarray, Wr: np.ndarray):
    """Host router: top-2 + softmax, fp64 logits for stable decisions."""
    logits = xf.astype(np.float64) @ Wr.astype(np.float64).T  # [N, E]
    top2 = np.argsort(-logits, axis=1, kind="stable")[:, :TOPK]  # [N, 2] desc
    lv = np.take_along_axis(logits, top2, axis=1).astype(np.float32)
    m = lv.max(axis=1, keepdims=True)
    ex = np.exp(lv - m)
    w = (ex / ex.sum(axis=1, keepdims=True)).astype(np.float32)  # [N, 2]
    return top2, w


def _split8(a: np.ndarray, scale: float):
    """hi/lo e4m3 split at a shared (power-of-2) scale."""
    s = (a * scale).astype(np.float32)
    hi = s.astype(E4NP)
    lo = (s - hi.astype(np.float32)).astype(E4NP)
    return hi, lo


def _pack_x(x8: np.ndarray, idx: np.ndarray, C: int) -> np.ndarray:
    """[C_e, 1024] e4m3 rows -> [KJ1, 128, 2, C] pair layout."""
    a = np.zeros((C, D), dtype=E4NP)
    a[: len(idx)] = x8[idx]
    # d = j*256 + i*128 + p
    return np.ascontiguousarray(a.T.reshape(KJ1, 2, 128, C).transpose(0, 2, 1, 3))


def _pack_w1(w: np.ndarray) -> np.ndarray:
    """[1024, 4096] e4m3 -> [4*KJ1, 128, 2, 1024] (f-quarter-major pairs)."""
    a = w.reshape(KJ1, 2, 128, 4, 1024).transpose(3, 0, 2, 1, 4)
    return np.ascontiguousarray(a.reshape(4 * KJ1, 128, 2, 1024))


def _pack_w2(w: np.ndarray) -> np.ndarray:
    """[4096, 1024] e4m3 -> [KJ2, 128, 2, 1024] pair layout."""
    return np.ascontiguousarray(w.reshape(KJ2, 2, 128, 1024).transpose(0, 2, 1, 3))


# SBUF budget: h tiles are 64*CP+32*CQ B/partition + ~110KB fixed.
C_SBUF_MAX = 1200


def _unpack_y(res_e, nm, Cn, ne):
    CBn, Rn = Cn // 128, Cn % 128
    parts = []
    if CBn:
        parts.append(res_e[f"y_{nm}"].astype(np.float32))
    if Rn:
        yre = res_e[f"yr_{nm}"].astype(np.float32)  # [8, 128, Rn]
        parts.append(yre.transpose(2, 0, 1).reshape(Rn, 1024))
    y = parts[0] if len(parts) == 1 else np.concatenate(parts, axis=0)
    return y[:ne]


def _run_pass(x8h, x8l, W1p, W2p, cls, out, trace):
    """One SPMD dispatch over the given per-expert token lists."""
    idxP, wtsP, idxQ, wtsQ = cls
    CP = max(256, (max(len(t) for t in idxP) + 15) // 16 * 16)
    CQ = max(64, (max(len(t) for t in idxQ) + 15) // 16 * 16)

    key = (CP, CQ)
    if key not in _cache:
        _cache[key] = _build(CP, CQ)
    nc = _cache[key]

    in_maps = []
    for e in range(E):
        xhl = np.concatenate(
            [
                _pack_x(x8h, idxP[e], CP),
                _pack_x(x8l, idxP[e], CP),
                _pack_x(x8h, idxQ[e], CQ),
            ],
            axis=3,
        )
        in_maps.append({"xhl": xhl, "w1hl": W1p[e], "w2hl": W2p[e]})

    res = run_bass_kernel_spmd(nc, in_maps, list(range(N_CORES)), trace=trace)

    for e in range(E):
        yep = _unpack_y(res.results[e], "p", CP, len(idxP[e]))
        out[idxP[e]] += (wtsP[e] / S_W2)[:, None] * yep
        yeq = _unpack_y(res.results[e], "q", CQ, len(idxQ[e]))
        out[idxQ[e]] += (wtsQ[e] / S_W2)[:, None] * yeq
    return res


def _run(x, Wr, W1, W2, trace=False):
    xf = np.asarray(x, dtype=np.float32).reshape(-1, D)
    N = xf.shape[0]
    top2, tw = _route(xf, np.asarray(Wr, dtype=np.float32))

    # host-side quantization (scales are powers of 2 -> exact descale)
    x8h, x8l = _split8(xf, 1.0)
    W1p, W2p = [], []
    for e in range(E):
        h1, l1 = _split8(np.asarray(W1[e], np.float32), S_W1)
        W1p.append(np.concatenate([_pack_w1(h1), _pack_w1(l1)], axis=3))
        h2, l2 = _split8(np.asarray(W2[e], np.float32), S_W2)
        W2p.append(np.concatenate([_pack_w2(h2), _pack_w2(l2)], axis=3))

    idxP, wtsP, idxQ, wtsQ = [], [], [], []
    for e in range(E):
        toks, ws = [], []
        for k in range(TOPK):
            tok = np.nonzero(top2[:, k] == e)[0]
            toks.append(tok)
            ws.append(tw[tok, k])
        tok = np.concatenate(toks)
        w = np.concatenate(ws).astype(np.float32)
        cheap = w < TAU
        idxP.append(tok[~cheap])
        wtsP.append(w[~cheap])
        idxQ.append(tok[cheap])
        wtsQ.append(w[cheap])

    # free accuracy: the precise capacity is set by the max-loaded expert, so
    # other experts have idle precise slots -- promote their highest-weight
    # cheap pairs into them (no change in compiled shapes or PE time).
    cap = max(256, (max(len(t) for t in idxP) + 15) // 16 * 16)
    for e in range(E):
        k = cap - len(idxP[e])
        if k <= 0 or len(idxQ[e]) == 0:
            continue
        k = min(k, len(idxQ[e]))
        order = np.argsort(-wtsQ[e], kind="stable")
        pro, keep = order[:k], order[k:]
        idxP[e] = np.concatenate([idxP[e], idxQ[e][pro]])
        wtsP[e] = np.concatenate([wtsP[e], wtsQ[e][pro]])
        idxQ[e] = idxQ[e][keep]
        wtsQ[e] = wtsQ[e][keep]

    cmax = max(max(len(t) for t in idxP), max(len(t) for t in idxQ))
    n_pass = max(1, math.ceil(cmax / C_SBUF_MAX))

    out = np.zeros((N, D), dtype=np.float32)
    res = None
    for p in range(n_pass):
        cls = tuple(
            [t[p * len(t) // n_pass : (p + 1) * len(t) // n_pass] for t in lst]
            for lst in (idxP, wtsP, idxQ, wtsQ)
        )
        res = _run_pass(x8h, x8l, W1p, W2p, cls, out, trace)
    return out.reshape(B, T, D), res


def kernel(x, Wr, W1, W2):
    out, _ = _run(x, Wr, W1, W2, trace=False)
    return out


# revision 41
# speedup vs baseline: 1.0013x; 1.0013x over previous
"""MoE feed-forward block (B=2, T=2048, D=1024, FF=4096, E=8, top-2) on 8 trn2 cores.

Strategy (expert-parallel, matching the sharding hint):
  - Router (x @ Wr.T, top-2, softmax) computed on host in fp64: it is tiny
    and its output is *indices* + weights, i.e. the dispatch.
  - Dispatch: tokens are gathered per expert on host (the all-to-all), padded
    to a common capacity, and each of the 8 cores runs the FFN of one expert
    over its routed tokens.
  - Combine: host does out[idx_e] += w_e * y_e (fp32), the weighted
    scatter-add, then reshapes to [B, T, D].

Device kernel: both GEMMs run on the PE in fp8 (e4m3) DoubleRow perf mode,
which contracts K=256 per instruction at 0.5 cycles/row -- 4x the fp16 MAC
rate. Plain e4m3 quantization costs ~2.7% error per operand, so (token,
expert) pairs are split into two precision classes by their softmax combine
weight w:

  precise (w >= TAU): each GEMM uses a compensated 3-product split
      A @ B ~= A_hi @ B_hi + A_lo @ B_hi + A_hi @ B_lo   (hi/lo both e4m3,
      shared power-of-2 scale, lo = quantized residual; ~7e-4 per operand)
      at 0.75x the fp16 cycle count.
  cheap (w < TAU): single-product pure fp8 at 0.25x the fp16 cycle count.
      Its ~5% FFN error enters the output scaled by w < TAU, and only a
      ~quarter of pairs land here, so the end-to-end error stays ~1.2e-2,
      under the 2e-2 gate.

Layouts (pair dim = the DoubleRow K-pair, i.e. k-blocks 2j/2j+1):
  GEMM1 (h = gelu(x @ W1)): psum[f128, ctile] += W1p[j][128,2,f128].T xp[j]
    [128,2,ctile]. Precise: 4 j-tiles x 3 products; ACT gelu -> fp16 h16,
    Pool casts h_hi (e4m3), DVE forms h_lo = h16 - h_hi. Cheap: 4 matmuls,
    ACT gelu straight to e4m3 hq. W1 scaled by 1024 on host; descaled by
    the ACT `scale` operand.
  GEMM2 (y = h @ W2): psum[c128, dtile] += hp[j2][128,2,c128].T W2p[j2]
    [128,2,dtile]; precise 3 products, cheap 1. The h pair APs slice
    per-c-chunk [128, 32, clen] SBUF tiles (pair stride = clen; per-chunk
    tiles keep the pair-read dependency boxes inside one chunk). W2 scaled
    by 2048 on host; descale folded into the host-side combine weights.
  Token-count remainders (CP%128, CQ%128) run transposed (W2 stationary, h
  moving, out [d128, R]) so their matmul cost scales with R, not 128.

DMA: merged hi|lo(|cheap) tensors keep the DMA count low (HWDGE descriptor
generation, ~625ns per DMA, is a serial resource). W1 streams on the SP
queue in consumption order, W2 follows; x rides the ACT hwdge queue with
the warmup-gating piece queued last, so the unavoidable head stall (the
early working set exceeds DMA bandwidth) collapses into one gap instead of
several -- each separate PE idle gap would restart the p-state ramp (3us at
half clock). GEMM2 interleaves its many small remainder/cheap groups
between big precise blocks, and the final store is split three ways so the
copy/DGE/DMA/sem tail chain mostly overlaps matmuls.
"""

import sys

sys.path.insert(0, "/opt/trn_rl_repo")

import math
from contextlib import ExitStack

import numpy as np
import ml_dtypes

import concourse.tile as tile
from concourse import bacc, mybir
from concourse.bass_utils import run_bass_kernel_spmd

B, T, D, FF, E, TOPK = 2, 2048, 1024, 4096, 8, 2
N_CORES = 8
FC = FF // 128  # 32 f-blocks
KJ1 = D // 256  # 4 K-pair tiles in GEMM1
KJ2 = FF // 256  # 16 K-pair tiles in GEMM2
S_W1 = 1024.0  # host scale on W1 (power of 2: exact)
S_W2 = 2048.0  # host scale on W2
TAU = 0.35  # combine-weight threshold for the cheap (pure-fp8) class

E4NP = ml_dtypes.float8_e4m3

_cache: dict[tuple, object] = {}


def _c_chunks(C: int) -> list[tuple[int, int]]:
    """Split C into <=512-wide moving chunks."""
    out, off = [], 0
    while off < C:
        n = min(512, C - off)
        out.append((off, n))
        off += n
    return out


def _build(CP: int, CQ: int):
    f16 = mybir.dt.float16
    e4 = mybir.dt.float8e4

    nc = bacc.Bacc("TRN2", target_bir_lowering=False, debug=False)
    # merged tensors (one DMA each -- HWDGE descriptor generation at
    # ~625ns per DMA is a serial resource, so fewer/bigger transfers win):
    # x pairs [j, p, i, (hi CP | lo CP | cheap CQ)], [c] = x[c, (2j+i)*128+p]
    xhl = nc.dram_tensor("xhl", [KJ1, 128, 2, 2 * CP + CQ], e4, kind="ExternalInput").ap()
    # W1 pairs, f-quarter-major [(q*4+j), p, i, (hi 1024 | lo 1024)],
    # [f'] = 1024*W1[(2j+i)*128+p, q*1024+f']
    w1hl = nc.dram_tensor("w1hl", [4 * KJ1, 128, 2, 2048], e4, kind="ExternalInput").ap()
    # W2 pairs [j2, p, i, (hi 1024 | lo 1024)], [d] = 2048*W2[(2j2+i)*128+p, d]
    w2hl = nc.dram_tensor("w2hl", [KJ2, 128, 2, 2048], e4, kind="ExternalInput").ap()

    outs = {}
    for nm, Cn in (("p", CP), ("q", CQ)):
        CBn, Rn = Cn // 128, Cn % 128
        outs[f"y_{nm}"] = (
            nc.dram_tensor(f"y_{nm}", [CBn * 128, 1024], f16, kind="ExternalOutput").ap()
            if CBn
            else None
        )
        outs[f"yr_{nm}"] = (
            nc.dram_tensor(f"yr_{nm}", [8, 128, Rn], f16, kind="ExternalOutput").ap()
            if Rn
            else None
        )

    with tile.TileContext(nc) as tc:
        _emit(nc, tc, xhl, w1hl, w2hl, outs, CP, CQ)
    nc.compile()
    return nc


def _emit(nc, tc, xhl, w1hl, w2hl, outs, CP, CQ):
    f16 = mybir.dt.float16
    f32 = mybir.dt.float32
    e4 = mybir.dt.float8e4
    GELU = mybir.ActivationFunctionType.Gelu
    COPY = mybir.ActivationFunctionType.Copy
    DR = mybir.MatmulPerfMode.DoubleRow
    chunksP = _c_chunks(CP)
    chunksQ = _c_chunks(CQ)
    CPB, RP = CP // 128, CP % 128
    CQB, RQ = CQ // 128, CQ % 128
    # chunk holding each class's token remainder (never straddles: chunk
    # edges and CB*128 are both 128-aligned)
    ciRP = (CPB * 128) // 512 if RP else -1
    ciRQ = (CQB * 128) // 512 if RQ else -1

    with ExitStack() as ctx:
        xp = ctx.enter_context(tc.tile_pool(name="xp", bufs=1))
        # a full f-quarter (hi+lo merged x 4 k-tiles) must be live at
        # once; 6 bufs gives half a quarter of prefetch. Merged tiles keep
        # the DMA count low (16): HWDGE descriptor generation (~625ns per
        # DMA) is a serial resource and limits W1 supply, not bandwidth.
        w1p = ctx.enter_context(tc.tile_pool(name="w1p", bufs=8))
        w2p = ctx.enter_context(tc.tile_pool(name="w2p", bufs=1))
        hp = ctx.enter_context(tc.tile_pool(name="hp", bufs=1))
        h16p = ctx.enter_context(tc.tile_pool(name="h16p", bufs=6))
        ps1p = ctx.enter_context(tc.tile_pool(name="ps1p", bufs=5, space="PSUM"))
        ps2p = ctx.enter_context(tc.tile_pool(name="ps2p", bufs=3, space="PSUM"))
        yp = ctx.enter_context(tc.tile_pool(name="yp", bufs=4))

        # --- input DMA. W1 streams alone on the SP queue in consumption
        # order; x rides the ACT hwdge queue; W2 follows W1 on SP. The j=0
        # x tile splits so the warmup's hi chunk leads.
        XW = 2 * CP + CQ
        x_t = [xp.tile([128, 2, XW], e4, name=f"x{j}") for j in range(KJ1)]
        c0len = chunksP[0][1]
        # j=0 (which gates the first warmup matmul group) goes LAST among
        # the early tiles: the PE start is delayed until the whole early
        # working set is nearly resident, consolidating the unavoidable
        # DMA-deficit stall into ONE gap -- each separate PE idle gap would
        # otherwise reset the p-state ramp (3us at half clock).
        for j in range(1, KJ1):
            nc.scalar.dma_start(x_t[j][:], xhl[j])
        nc.scalar.dma_start(x_t[0][:, :, :c0len], xhl[0][:, :, :c0len])
        nc.scalar.dma_start(x_t[0][:, :, c0len:], xhl[0][:, :, c0len:])

        w1_t = {}
        for q in range(4):
            for j in range(KJ1):
                t = w1p.tile([128, 2, 2048], e4, tag="w1", name=f"w1_{q}_{j}")
                nc.sync.dma_start(t[:], w1hl[q * KJ1 + j])
                w1_t[q, j] = t

        w2_t = []
        for j2 in range(KJ2):
            t = w2p.tile([128, 2, 2048], e4, name=f"w2_{j2}")
            nc.sync.dma_start(t[:], w2hl[j2])
            w2_t.append(t)

        # per-c-chunk h tiles (pair-read dependency boxes stay in-chunk)
        hh_c = [hp.tile([128, FC, cl], e4, name=f"hh{ci}") for ci, (_, cl) in enumerate(chunksP)]
        hl_c = [hp.tile([128, FC, cl], e4, name=f"hl{ci}") for ci, (_, cl) in enumerate(chunksP)]
        hq_c = [hp.tile([128, FC, cl], e4, name=f"hq{ci}") for ci, (_, cl) in enumerate(chunksQ)]

        def w1_slices(fb, j, lo=False):
            t = w1_t[fb // 8, j]
            off = (1024 if lo else 0) + (fb % 8) * 128
            return t[:, :, off : off + 128]

        def g1_precise(ps, fb, coff, clen, j, first, last, skip_lo=False):
            lh = w1_slices(fb, j)
            ll = w1_slices(fb, j, lo=True)
            rh = x_t[j][:, :, coff : coff + clen]
            rl = x_t[j][:, :, CP + coff : CP + coff + clen]
            o = ps[:, :clen]
            nc.tensor.matmul(o, lh, rh, start=first, stop=False, perf_mode=DR)
            nc.tensor.matmul(o, ll, rh, start=False, stop=False, perf_mode=DR)
            if not skip_lo:
                nc.tensor.matmul(o, lh, rl, start=False, stop=last, perf_mode=DR)

        def g1_post(ps, fb, ci, clen):
            # one ACT gelu pass (fp16); Pool casts the hi part to e4m3;
            # DVE forms the residual. Spreads the work over three engines.
            h16 = h16p.tile([128, 512], f16, tag="h16", name=f"h16_{fb}_{ci}")
            nc.scalar.activation(h16[:, :clen], ps[:, :clen], GELU, scale=1.0 / S_W1)
            nc.gpsimd.tensor_copy(hh_c[ci][:, fb, :clen], h16[:, :clen])
            nc.vector.tensor_sub(
                hl_c[ci][:, fb, :clen], h16[:, :clen], hh_c[ci][:, fb, :clen]
            )

        def g1_cheap(fb):
            for ci, (coff, clen) in enumerate(chunksQ):
                ps = ps1p.tile([128, 512], f32, tag="ps1", name=f"psq_{fb}_{ci}")
                o = ps[:, :clen]
                for j in range(KJ1):
                    rq = x_t[j][:, :, 2 * CP + coff : 2 * CP + coff + clen]
                    nc.tensor.matmul(
                        o, w1_slices(fb, j), rq,
                        start=(j == 0), stop=(j == KJ1 - 1), perf_mode=DR,
                    )
                nc.scalar.activation(hq_c[ci][:, fb, :clen], o, GELU, scale=1.0 / S_W1)

        # --- GEMM1. Warmup: j-outer over the 4 f-blocks of half-quarter 0,
        # precise chunk 0, hi-products first (so only x_hi gates the start).
        warm_fb = 4
        coff0 = chunksP[0][0]
        ps_head = [
            ps1p.tile([128, 512], f32, tag="ps1", name=f"psh_{fb}")
            for fb in range(warm_fb)
        ]
        jorder = [1, 2, 3, 0]  # matches x-tile arrival order
        for ji, j in enumerate(jorder):
            for fb in range(warm_fb):
                g1_precise(ps_head[fb], fb, coff0, c0len, j,
                           first=(ji == 0), last=False, skip_lo=True)
        for ji, j in enumerate(jorder):
            for fb in range(warm_fb):
                rl = x_t[j][:, :, CP + coff0 : CP + coff0 + c0len]
                nc.tensor.matmul(
                    ps_head[fb][:, :c0len], w1_slices(fb, j), rl,
                    start=False, stop=(ji == KJ1 - 1), perf_mode=DR,
                )
        for fb in range(warm_fb):
            g1_post(ps_head[fb], fb, 0, c0len)

        # regular groups fb-major (= W1 stream order). Per fb: the cheap
        # group first (GEMM2's cheap phase runs early), then precise chunks
        # with the remainder-holding chunk first.
        idx_chunks = list(enumerate(chunksP))
        if 0 <= ciRP and len(chunksP) > 1:
            idx_chunks = [idx_chunks[ciRP]] + idx_chunks[:ciRP] + idx_chunks[ciRP + 1 :]
        for fb in range(FC):
            g1_cheap(fb)
            for ci, (coff, clen) in idx_chunks:
                if fb < warm_fb and ci == 0:
                    continue
                ps = ps1p.tile([128, 512], f32, tag="ps1", name=f"ps1_{fb}_{ci}")
                for j in range(KJ1):
                    g1_precise(ps, fb, coff, clen, j,
                               first=(j == 0), last=(j == KJ1 - 1))
                g1_post(ps, fb, ci, clen)

        # --- GEMM2. Phase order puts work whose h deps complete earliest
        # first, so the PE always has queued work while the GEMM1 ACT/Pool/
        # DVE pipeline drains: precise remainder -> cheap (blocks+remainder)
        # -> precise blocks in the remainder's chunk -> remaining blocks.
        def rem_db(hh_src, hl_src, ci, lo, R, yr, n_prod, db):
            # transposed: W2 stationary, h moving, out [d-block 128, R]
            ps = ps2p.tile([128, 512], f32, tag="ps2", name=f"psr_{db}_{n_prod}")
            o = ps[:, :R]
            for j2 in range(KJ2):
                t2 = w2_t[j2]
                lh = t2[:, :, db * 128 : (db + 1) * 128]
                ll = t2[:, :, 1024 + db * 128 : 1024 + (db + 1) * 128]
                rh = hh_src[ci][:, 2 * j2 : 2 * j2 + 2, lo : lo + R]
                nc.tensor.matmul(o, lh, rh, start=(j2 == 0),
                                 stop=(n_prod == 1 and j2 == KJ2 - 1), perf_mode=DR)
                if n_prod == 3:
                    rl = hl_src[ci][:, 2 * j2 : 2 * j2 + 2, lo : lo + R]
                    nc.tensor.matmul(o, ll, rh, start=False, stop=False, perf_mode=DR)
                    nc.tensor.matmul(o, lh, rl, start=False,
                                     stop=(j2 == KJ2 - 1), perf_mode=DR)
            yrs = yp.tile([128, R], f16, tag=f"yr{n_prod}", name=f"yr_{db}_{n_prod}", bufs=2)
            # ACT (idle once GEMM1's gelus drain) does these copies; DVE is
            # still working off the GEMM1 residual-sub queue.
            nc.scalar.activation(yrs[:], ps[:, :R], COPY)
            nc.sync.dma_start(yr[db], yrs[:])

        def g2_block(hh_src, hl_src, y_dst, chunks_, cb, doff, dlen, n_prod, tag):
            ci = (cb * 128) // 512
            cl = cb * 128 - chunks_[ci][0]
            ps = ps2p.tile([128, 512], f32, tag="ps2", name=f"ps2_{cb}_{doff}_{n_prod}{tag}")
            o = ps[:, :dlen]
            for j2 in range(KJ2):
                t2 = w2_t[j2]
                lh = hh_src[ci][:, 2 * j2 : 2 * j2 + 2, cl : cl + 128]
                rh = t2[:, :, doff : doff + dlen]
                nc.tensor.matmul(o, lh, rh, start=(j2 == 0),
                                 stop=(n_prod == 1 and j2 == KJ2 - 1), perf_mode=DR)
                if n_prod == 3:
                    ll = hl_src[ci][:, 2 * j2 : 2 * j2 + 2, cl : cl + 128]
                    rl = t2[:, :, 1024 + doff : 1024 + doff + dlen]
                    nc.tensor.matmul(o, ll, rh, start=False, stop=False, perf_mode=DR)
                    nc.tensor.matmul(o, lh, rl, start=False,
                                     stop=(j2 == KJ2 - 1), perf_mode=DR)
            ysb = yp.tile([128, 512], f16, tag="y", name=f"y_{cb}_{doff}_{n_prod}{tag}")
            nc.vector.tensor_copy(ysb[:, :dlen], o)
            nc.sync.dma_start(y_dst[cb * 128 : (cb + 1) * 128, doff : doff + dlen], ysb[:, :dlen])

        # Emission order: two big precise blocks (whose h deps complete by
        # GEMM1's end) lead; the many small remainder/cheap groups are then
        # interleaved ~2 per big block so their store-chain latencies hide
        # behind 5us of queued matmuls; the split final store goes last.
        smalls = []
        if RP:
            loP = CPB * 128 - chunksP[ciRP][0]
            smalls += [
                (lambda db=db: rem_db(hh_c, hl_c, ciRP, loP, RP, outs["yr_p"], 3, db))
                for db in range(8)
            ]
        for cqb in range(CQB):
            for doff in (0, 512):
                smalls.append(
                    lambda cqb=cqb, doff=doff: g2_block(
                        hq_c, None, outs["y_q"], chunksQ, cqb, doff, 512, 1, ""
                    )
                )
        if RQ:
            loQ = CQB * 128 - chunksQ[ciRQ][0]
            smalls += [
                (lambda db=db: rem_db(hq_c, None, ciRQ, loQ, RQ, outs["yr_q"], 1, db))
                for db in range(8)
            ]

        cbs = sorted(range(CPB), key=lambda cb: (0 if (cb * 128) // 512 == ciRP else 1, cb))
        bigs = []
        for i, cb in enumerate(cbs):
            for doff in (0, 512):
                if i == len(cbs) - 1 and doff == 512:
                    bigs.append(
                        lambda cb=cb: (
                            g2_block(hh_c, hl_c, outs["y_p"], chunksP, cb, 512, 256, 3, "a"),
                            g2_block(hh_c, hl_c, outs["y_p"], chunksP, cb, 768, 128, 3, "b"),
                            g2_block(hh_c, hl_c, outs["y_p"], chunksP, cb, 896, 128, 3, "c"),
                        )
                    )
                else:
                    bigs.append(
                        lambda cb=cb, doff=doff: g2_block(
                            hh_c, hl_c, outs["y_p"], chunksP, cb, doff, 512, 3, ""
                        )
                    )
        n_lead = min(2, len(bigs) - 1)
        for bg in bigs[:n_lead]:
            bg()
        rest = bigs[n_lead:]
        si = 0
        for k, bg in enumerate(rest):
            # front-load the smalls (3 per big) so the late phase is pure
            # big groups with no small-store chain latencies interspersed
            for _ in range(3):
                if si < len(smalls):
                    smalls[si]()
                    si += 1
            bg()
        while si < len(smalls):
            smalls[si]()
            si += 1


def _route(xf: np.ndarray, Wr: np.ndarray):
    """Host router: top-2 + softmax, fp64 logits for stable decisions."""
    logits = xf.astype(np.float64) @ Wr.astype(np.float64).T  # [N, E]
    top2 = np.argsort(-logits, axis=1, kind="stable")[:, :TOPK]  # [N, 2] desc
    lv = np.take_along_axis(logits, top2, axis=1).astype(np.float32)
    m = lv.max(axis=1, keepdims=True)
    ex = np.exp(lv - m)
    w = (ex / ex.sum(axis=1, keepdims=True)).astype(np.float32)  # [N, 2]
    return top2, w


def _split8(a: np.ndarray, scale: float):
    """hi/lo e4m3 split at a shared (power-of-2) scale."""
    s = (a * scale).astype(np.float32)
    hi = s.astype(E4NP)
    lo = (s - hi.astype(np.float32)).astype(E4NP)
    return hi, lo


def _pack_x(x8: np.ndarray, idx: np.ndarray, C: int) -> np.ndarray:
    """[C_e, 1024] e4m3 rows -> [KJ1, 128, 2, C] pair layout."""
    a = np.zeros((C, D), dtype=E4NP)
    a[: len(idx)] = x8[idx]
    # d = j*256 + i*128 + p
    return np.ascontiguousarray(a.T.reshape(KJ1, 2, 128, C).transpose(0, 2, 1, 3))


def _pack_w1(w: np.ndarray) -> np.ndarray:
    """[1024, 4096] e4m3 -> [4*KJ1, 128, 2, 1024] (f-quarter-major pairs)."""
    a = w.reshape(KJ1, 2, 128, 4, 1024).transpose(3, 0, 2, 1, 4)
    return np.ascontiguousarray(a.reshape(4 * KJ1, 128, 2, 1024))


def _pack_w2(w: np.ndarray) -> np.ndarray:
    """[4096, 1024] e4m3 -> [KJ2, 128, 2, 1024] pair layout."""
    return np.ascontiguousarray(w.reshape(KJ2, 2, 128, 1024).transpose(0, 2, 1, 3))


# SBUF budget: h tiles are 64*CP+32*CQ B/partition + ~110KB fixed.
C_SBUF_MAX = 1200


def _unpack_y(res_e, nm, Cn, ne):
    CBn, Rn = Cn // 128, Cn % 128
    parts = []
    if CBn:
        parts.append(res_e[f"y_{nm}"].astype(np.float32))
    if Rn:
        yre = res_e[f"yr_{nm}"].astype(np.float32)  # [8, 128, Rn]
        parts.append(yre.transpose(2, 0, 1).reshape(Rn, 1024))
    y = parts[0] if len(parts) == 1 else np.concatenate(parts, axis=0)
    return y[:ne]


def _run_pass(x8h, x8l, W1p, W2p, cls, out, trace):
    """One SPMD dispatch over the given per-expert token lists."""
    idxP, wtsP, idxQ, wtsQ = cls
    CP = max(256, (max(len(t) for t in idxP) + 15) // 16 * 16)
    CQ = max(64, (max(len(t) for t in idxQ) + 15) // 16 * 16)

    key = (CP, CQ)
    if key not in _cache:
        _cache[key] = _build(CP, CQ)
    nc = _cache[key]

    in_maps = []
    for e in range(E):
        xhl = np.concatenate(
            [
                _pack_x(x8h, idxP[e], CP),
                _pack_x(x8l, idxP[e], CP),
                _pack_x(x8h, idxQ[e], CQ),
            ],
            axis=3,
        )
        in_maps.append({"xhl": xhl, "w1hl": W1p[e], "w2hl": W2p[e]})

    res = run_bass_kernel_spmd(nc, in_maps, list(range(N_CORES)), trace=trace)

    for e in range(E):
        yep = _unpack_y(res.results[e], "p", CP, len(idxP[e]))
        out[idxP[e]] += (wtsP[e] / S_W2)[:, None] * yep
        yeq = _unpack_y(res.results[e], "q", CQ, len(idxQ[e]))
        out[idxQ[e]] += (wtsQ[e] / S_W2)[:, None] * yeq
    return res


def _run(x, Wr, W1, W2, trace=False):
    xf = np.asarray(x, dtype=np.float32).reshape(-1, D)
    N = xf.shape[0]
    top2, tw = _route(xf, np.asarray(Wr, dtype=np.float32))

    # host-side quantization (scales are powers of 2 -> exact descale)
    x8h, x8l = _split8(xf, 1.0)
    W1p, W2p = [], []
    for e in range(E):
        h1, l1 = _split8(np.asarray(W1[e], np.float32), S_W1)
        W1p.append(np.concatenate([_pack_w1(h1), _pack_w1(l1)], axis=3))
        h2, l2 = _split8(np.asarray(W2[e], np.float32), S_W2)
        W2p.append(np.concatenate([_pack_w2(h2), _pack_w2(l2)], axis=3))

    idxP, wtsP, idxQ, wtsQ = [], [], [], []
    for e in range(E):
        toks, ws = [], []
        for k in range(TOPK):
            tok = np.nonzero(top2[:, k] == e)[0]
            toks.append(tok)
            ws.append(tw[tok, k])
        tok = np.concatenate(toks)
        w = np.concatenate(ws).astype(np.float32)
        cheap = w < TAU
        idxP.append(tok[~cheap])
        wtsP.append(w[~cheap])
        idxQ.append(tok[cheap])
        wtsQ.append(w[cheap])

    # free accuracy: the precise capacity is set by the max-loaded expert, so
    # other experts have idle precise slots -- promote their highest-weight
    # cheap pairs into them (no change in compiled shapes or PE time).
    cap = max(256, (max(len(t) for t in idxP) + 15) // 16 * 16)
    for e in range(E):
        k = cap - len(idxP[e])
        if k <= 0 or len(idxQ[e]) == 0:
            continue
        k = min(k, len(idxQ[e]))
        order = np.argsort(-wtsQ[e], kind="stable")
        pro, keep = order[:k], order[k:]
        idxP[e] = np.concatenate([idxP[e], idxQ[e][pro]])
        wtsP[e] = np.concatenate([wtsP[e], wtsQ[e][pro]])
        idxQ[e] = idxQ[e][keep]
        wtsQ[e] = wtsQ[e][keep]

    cmax = max(max(len(t) for t in idxP), max(len(t) for t in idxQ))
    n_pass = max(1, math.ceil(cmax / C_SBUF_MAX))

    out = np.zeros((N, D), dtype=np.float32)
    res = None
    for p in range(n_pass):
        cls = tuple(
            [t[p * len(t) // n_pass : (p + 1) * len(t) // n_pass] for t in lst]
            for lst in (idxP, wtsP, idxQ, wtsQ)
        )
        res = _run_pass(x8h, x8l, W1p, W2p, cls, out, trace)
    return out.reshape(B, T, D), res


def kernel(x, Wr, W1, W2):
    out, _ = _run(x, Wr, W1, W2, trace=False)
    return out
